# revision 44
# baseline (speedup 1.0000x reference)
"""Bimamba (bidirectional Mamba) block on 8 trn2 NeuronCores.

Sharding: tensor-parallel over d_inner (256 channels/core). LayerNorm is
computed redundantly per core on full-token f16 inputs (no AllGather);
x_proj partial sums are AllReduced per direction in f16; out_proj is
resolved with two m-split token AllToAlls so the first half overlaps the
final scan block. The selective scan runs b-merged [128, 4096] via a
poison-column decay reset at the batch boundary.
"""
import sys, os, json, time

sys.path.insert(0, '/opt/trn_rl_repo')

import numpy as np
import concourse.bass as bass
import concourse.mybir as mybir
import concourse.tile as tile
import bass_rust
from concourse.vector_clock import ScopedClock
from concourse import bass2jax
import jax

# ----------------------------------------------------------------- patches

def _patched_drain_and_barrier(self, tick_clock, wait_clock):
    nc = self.nc
    gc = tick_clock.global_clock
    vals = json.loads(repr(gc).replace("VectorClock(", "").rstrip(")"))
    procs = [i for i, v in enumerate(vals) if v > 0]
    for p in procs:
        sub = bass_rust.VectorClock()
        sub.require_at_least(p, vals[p])
        nop = nc.sync.nop(nofuse=True)
        wait_clock.add_sem_waits(nop.ins, ScopedClock({None: sub}))
    nc.sync.drain()
    nc.all_engine_barrier()
    assert self.sems is not None
    popped = nc._tile_sem_poison_stack.pop()
    assert popped is self._sem_poison
    nc.clear_and_free_semaphores(list(self.sems.allocated().values()))
    nc.all_engine_barrier()


tile.TileContext._drain_and_barrier = _patched_drain_and_barrier

_SPLIT_ENGINES = {"SP", "PE", "DVE", "Activation", "Pool"}
_wsplit_ctr = [0]


def _split_excess_waits(bir, max_waits=1):
    for f in bir.get("functions") or []:
        for blk in f.get("blocks") or []:
            insts = blk.get("instructions") or []
            out = []
            for inst in insts:
                si = inst.get("sync_info")
                waits = (si or {}).get("on_wait") or []
                eng = inst.get("engine")
                if len(waits) > max_waits and eng in _SPLIT_ENGINES:
                    keep, extra = waits[:max_waits], waits[max_waits:]
                    for i in range(0, len(extra), max_waits):
                        _wsplit_ctr[0] += 1
                        out.append({
                            "debug": inst.get("debug", 0),
                            "engine": eng,
                            "ins": [], "outs": [],
                            "name": f"WSPLIT-{_wsplit_ctr[0]}",
                            "opcode": "NoOp",
                            "sync_info": {"on_update": [],
                                          "on_wait": extra[i:i + max_waits]},
                        })
                    si["on_wait"] = keep
                out.append(inst)
            blk["instructions"] = out
    return bir


if not getattr(bass.Bass, "_ws_patched", False):
    _orig_to_json_bytes = bass.Bass.to_json_bytes

    def _patched_to_json_bytes(self):
        bir = json.loads(_orig_to_json_bytes(self))
        _split_excess_waits(bir)
        return json.dumps(bir).encode()

    bass.Bass.to_json_bytes = _patched_to_json_bytes
    bass.Bass._ws_patched = True

# ----------------------------------------------------------------- consts

B, D, L = 2, 1024, 2048
DIN, NST, DTR, KCV = 2048, 16, 64, 4
NC_ = 8
DL = DIN // NC_          # 256 channels per core
TOK = B * L              # 4096 tokens, b-major
TSL = TOK // NC_         # 512-token slice per core
EPS = 1e-5

f32 = mybir.dt.float32
f16 = mybir.dt.float16
AL = mybir.AluOpType
AF = mybir.ActivationFunctionType

NXP = DTR + 2 * NST      # 96
POISON = 60000.0         # dt poison at the b-boundary column: exp(A*POISON)=0

# which n-iterations route their C-mult to GpSimd (per scan block).
# Empirically net-negative on trn2: POOL shares an SBUF port with DVE and
# concurrent gp tensor_tensor slows the DVE scans ~15%. Keep empty.
GP_N = ()


# ----------------------------------------------------------------- program

def build_program(reps=1):
    nc = bass.Bass(trn_type="TRN2", target_bir_lowering=False, num_devices=NC_)

    def din(name, shape, dt=f32):
        return nc.dram_tensor(name, list(shape), dt, kind="ExternalInput").ap()

    def dout(name, shape, dt=f32):
        return nc.dram_tensor(name, list(shape), dt, kind="ExternalOutput").ap()

    hsf_in = din("hsf", (D, TOK), f16)       # full tokens, f16 (LN is redundant)
    resf_in = din("resf", (D, TOK), f16)
    hss_in = din("hss", (D, TSL))            # per-core f32 slices (exact r_out)
    ress_in = din("ress", (D, TSL))
    wx_in = din("wxT", (D, DL), f16)         # in_proj x-rows lhsT (gamma folded)
    wz_in = din("wzT", (D, DL), f16)
    w1_in = din("w1s", (1, 2 * DL), f16)     # -(row sums) for the mu rank-1 fold
    bx_in = din("bx", (DL, 1))               # in_proj beta-fold biases
    bz_in = din("bz", (DL, 1))
    cvd_in = din("convdiag", (2, KCV, 2, 128, 128), f16)   # (dir,tap,m,.,.)
    cb_in = din("convb", (2, DL, 1))
    xw_in = din("xwT", (2, DL, NXP), f16)    # (dir, k=dl, 96)
    dtw_in = din("dtwT", (2, DTR, DL), f16)
    dtb_in = din("dtb", (2, DL, 1))
    atab_in = din("atab", (2, DL, NST))
    dpd_in = din("dpdiag", (2, 2, 128, 128), f16)
    wop_in = din("wopT", (DIN, D), f16)
    opb_in = din("opb", (D, 1))
    i128_in = din("i128", (128, 128), f16)
    ones_in = din("ones", (128, 1), f16)

    r_out = dout("r_out", (D, TSL))          # per-core r token slice
    o_out = dout("o_out", (D, TSL))          # out token-slice (per core)

    with tile.TileContext(nc) as tc:
        with tc.tile_pool(name="wts", bufs=1) as wts, \
             tc.tile_pool(name="dram", bufs=1, space="DRAM") as dram:

            # ---- small weights; DMAs deferred until after LN input loads
            wload = []

            def wdma(t, s):
                wload.append((t, s))

            wx_sb = [wts.tile([128, DL], f16, tag=f"wx{k}", name=f"wx{k}") for k in range(8)]
            wz_sb = [wts.tile([128, DL], f16, tag=f"wz{k}", name=f"wz{k}") for k in range(8)]
            for k in range(8):
                wdma(wx_sb[k][:], wx_in[k * 128:(k + 1) * 128, :])
                wdma(wz_sb[k][:], wz_in[k * 128:(k + 1) * 128, :])
            bx_sb = [wts.tile([128, 1], f32, tag=f"bx{m}", name=f"bx{m}") for m in range(2)]
            bz_sb = [wts.tile([128, 1], f32, tag=f"bz{m}", name=f"bz{m}") for m in range(2)]
            for m in range(2):
                wdma(bx_sb[m][:], bx_in[m * 128:(m + 1) * 128, :])
                wdma(bz_sb[m][:], bz_in[m * 128:(m + 1) * 128, :])
            cvd_sb = {}
            for dr in range(2):
                for j in range(KCV):
                    for m in range(2):
                        t = wts.tile([128, 128], f16, tag=f"cv{dr}{j}{m}", name=f"cv{dr}{j}{m}")
                        wdma(t[:], cvd_in[dr, j, m])
                        cvd_sb[dr, j, m] = t
            cb_sb = {}
            dtb_sb = {}
            at_sb = {}
            dpd_sb = {}
            for dr in range(2):
                for m in range(2):
                    t = wts.tile([128, 1], f32, tag=f"cb{dr}{m}", name=f"cb{dr}{m}")
                    wdma(t[:], cb_in[dr, m * 128:(m + 1) * 128, :])
                    cb_sb[dr, m] = t
                    t = wts.tile([128, 1], f32, tag=f"db{dr}{m}", name=f"db{dr}{m}")
                    wdma(t[:], dtb_in[dr, m * 128:(m + 1) * 128, :])
                    dtb_sb[dr, m] = t
                    t = wts.tile([128, NST], f32, tag=f"at{dr}{m}", name=f"at{dr}{m}")
                    wdma(t[:], atab_in[dr, m * 128:(m + 1) * 128, :])
                    at_sb[dr, m] = t
                    t = wts.tile([128, 128], f16, tag=f"dp{dr}{m}", name=f"dp{dr}{m}")
                    wdma(t[:], dpd_in[dr, m])
                    dpd_sb[dr, m] = t
            xw_sb = {}
            for dr in range(2):
                for m in range(2):
                    t = wts.tile([128, NXP], f16, tag=f"xw{dr}{m}", name=f"xw{dr}{m}")
                    wdma(t[:], xw_in[dr, m * 128:(m + 1) * 128, :])
                    xw_sb[dr, m] = t
            dtw_sb = {}
            for dr in range(2):
                t = wts.tile([DTR, DL], f16, tag=f"dtw{dr}", name=f"dtw{dr}")
                wdma(t[:], dtw_in[dr])
                dtw_sb[dr] = t
            i128_sb = wts.tile([128, 128], f16, tag="i128", name="i128")
            wdma(i128_sb[:], i128_in)
            w1_sb = wts.tile([1, 2 * DL], f16, tag="w1s", name="w1s")
            wdma(w1_sb[:], w1_in)
            ones_sb = wts.tile([128, 1], f16, tag="ones", name="ones")
            nc.sync.dma_start(ones_sb[:], ones_in)
            opb_sb = [wts.tile([128, 1], f32, tag=f"opb{m}", name=f"opb{m}") for m in range(8)]
            for m in range(8):
                wdma(opb_sb[m][:], opb_in[m * 128:(m + 1) * 128, :])

            for _rep in range(reps):
                cm_zs = tc.tile_pool(name="zsp", bufs=1)
                zsp = cm_zs.__enter__()
                zs16 = [zsp.tile([128, TOK], f16, tag=f"zs{m}", name=f"zs{m}") for m in range(2)]
                cm_u = tc.tile_pool(name="up", bufs=1)
                upool = cm_u.__enter__()
                u16 = {(dr, m): upool.tile([128, TOK], f16, tag=f"u{dr}{m}", name=f"u{dr}{m}")
                       for dr in range(2) for m in range(2)}
                cm_dtA = tc.tile_pool(name="dtA", bufs=1)
                dtA = cm_dtA.__enter__()
                y16 = [None, None]   # allocated in the scan-block pool
                cm_xp = tc.tile_pool(name="xpp", bufs=1)
                xpp = cm_xp.__enter__()
                xpad = {(m, b): xpp.tile([128, L + 6], f16, tag=f"xp{m}{b}", name=f"xp{m}{b}")
                        for m in range(2) for b in range(2)}
                for m in range(2):
                    for b in range(2):
                        nc.vector.memset(xpad[m, b][:, 0:3], 0.0)
                        nc.vector.memset(xpad[m, b][:, L + 3:L + 6], 0.0)

                # ======== Phase A: redundant LN over all 4096 tokens =========
                # hn[k] holds r in f16, later normalized in place.
                cm_rhn = tc.tile_pool(name="rhn", bufs=1)
                rhn = cm_rhn.__enter__()
                hn = [rhn.tile([128, TOK], f16, tag=f"hn{k}", name=f"hn{k}")
                      for k in range(8)]
                with tc.tile_pool(name="lnw", bufs=1) as lnw, \
                     tc.tile_pool(name="lnps", bufs=1, space="PSUM") as lnps, \
                     tc.tile_pool(name="lnsm", bufs=1) as lnsm:
                    sst = [lnps.tile([33, 512], f32, tag=f"ss{c}", name=f"ss{c}")
                           for c in range(8)]
                    for k in range(8):
                        nc.sync.dma_start(hn[k][:], hsf_in[k * 128:(k + 1) * 128, :])
                        for hh in range(2):
                            hsl = slice(hh * L, (hh + 1) * L)
                            re_t = lnw.tile([128, L], f16, tag="re", name="re_t", bufs=2)
                            nc.sync.dma_start(
                                re_t[:], resf_in[k * 128:(k + 1) * 128, hsl])
                            nc.vector.tensor_tensor(hn[k][:, hsl], hn[k][:, hsl],
                                                    re_t[:], AL.add)
                            sq_t = lnw.tile([128, L], f16, tag="sqt", name="sq_t", bufs=1)
                            nc.scalar.activation(sq_t[:], hn[k][:, hsl], AF.Square)
                            for cc in range(4):
                                c = hh * 4 + cc
                                csl = slice(c * 512, (c + 1) * 512)
                                qsl = slice(cc * 512, (cc + 1) * 512)
                                nc.tensor.matmul(sst[c][0:1, :], ones_sb[:],
                                                 hn[k][:, csl],
                                                 start=(k == 0), stop=(k == 7))
                                nc.tensor.matmul(sst[c][32:33, :], ones_sb[:],
                                                 sq_t[:, qsl],
                                                 start=(k == 0), stop=(k == 7))
                    # exact r slice for r_out (f32 path, off critical path)
                    for k in range(8):
                        hs_s = lnw.tile([128, TSL], f32, tag="hss", name="hs_s", bufs=1)
                        re_s = lnw.tile([128, TSL], f32, tag="res", name="re_s", bufs=1)
                        nc.sync.dma_start(hs_s[:], hss_in[k * 128:(k + 1) * 128, :])
                        nc.sync.dma_start(re_s[:], ress_in[k * 128:(k + 1) * 128, :])
                        nc.vector.tensor_tensor(hs_s[:], hs_s[:], re_s[:], AL.add)
                        nc.sync.dma_start(r_out[k * 128:(k + 1) * 128, :], hs_s[:])
                    # stats rows gathered to [8, 512] (partition-parallel recip)
                    mu8 = lnsm.tile([8, 512], f16, tag="mu8", name="mu8")
                    ex8 = lnsm.tile([8, 512], f16, tag="ex8", name="ex8")
                    for c in range(8):
                        s1 = lnsm.tile([1, 512], f16, tag="s1", name="s1", bufs=2)
                        s2 = lnsm.tile([1, 512], f16, tag="s2", name="s2", bufs=2)
                        nc.vector.tensor_scalar_mul(s1[:], sst[c][0:1, :], 1.0 / D)
                        nc.vector.tensor_scalar_mul(s2[:], sst[c][32:33, :], 1.0 / D)
                        nc.sync.dma_start(mu8[c:c + 1, :], s1[:])
                        nc.sync.dma_start(ex8[c:c + 1, :], s2[:])
                    tmp8 = lnsm.tile([8, 512], f16, tag="tmp8", name="tmp8")
                    nc.vector.tensor_tensor(tmp8[:], mu8[:], mu8[:], AL.mult)
                    nc.vector.tensor_tensor(ex8[:], ex8[:], tmp8[:], AL.subtract)
                    nc.vector.tensor_scalar_add(ex8[:], ex8[:], float(EPS))
                    nc.scalar.activation(ex8[:], ex8[:], AF.Sqrt)
                    with nc.allow_low_precision(reason="LN stats in f16; tol 2e-2"):
                        nc.vector.reciprocal(tmp8[:], ex8[:])
                    r16_ = lnsm.tile([8, 512], f16, tag="r16_", name="r16_")
                    m16_ = lnsm.tile([8, 512], f16, tag="m16_", name="m16_")
                    nc.vector.tensor_copy(r16_[:], tmp8[:])
                    # murs = mu * rstd (folded into in_proj as a rank-1 term)
                    nc.vector.tensor_tensor(m16_[:], mu8[:], tmp8[:], AL.mult)
                    drow = dram.tile([2, TOK], f16, tag="stat", name="stat")
                    for c in range(8):
                        csl = slice(c * 512, (c + 1) * 512)
                        nc.sync.dma_start(drow[0:1, csl], r16_[c:c + 1, :])
                        nc.sync.dma_start(drow[1:2, csl], m16_[c:c + 1, :])
                    mursr = rhn.tile([1, TOK], f16, tag="mursr", name="mursr")
                    nc.sync.dma_start(mursr[:], drow[1:2, :])
                    for hh in range(2):
                        hsl = slice(hh * L, (hh + 1) * L)
                        rb = lnsm.tile([128, L], f16, tag="rbc", name="rbc", bufs=2)
                        nc.sync.dma_start(rb[:], drow[0:1, hsl].broadcast_to((128, L)))
                        for k in range(8):
                            nc.vector.tensor_tensor(hn[k][:, hsl], hn[k][:, hsl],
                                                    rb[:], AL.mult)

                if _rep == 0:
                    for _t, _s in wload[:20]:   # wx/wz/bx/bz: in_proj needs these
                        nc.sync.dma_start(_t, _s)

                # ======== Phase B/C: in_proj-x, conv0, xproj0, AR0, z, ... ===
                ar_src = [dram.tile([NXP, TOK], f16, tag=f"ars{dr}", name=f"ars{dr}")
                          for dr in range(2)]
                ar_dst = [dram.tile([NXP, TOK], f16, tag=f"ard{dr}", name=f"ard{dr}",
                                    addr_space="Shared") for dr in range(2)]

                def in_proj_half(hps, m):
                    # m in 0..3: 0,1 = x halves; 2,3 = z halves
                    for ch in range(8):
                        ps = hps.tile([128, 512], f32, tag="ps", name="ps")
                        for k in range(8):
                            w = wx_sb[k] if m < 2 else wz_sb[k]
                            lh = w[:, (m % 2) * 128:(m % 2) * 128 + 128]
                            nc.tensor.matmul(ps[:], lh,
                                             hn[k][:, ch * 512:(ch + 1) * 512],
                                             start=(k == 0), stop=False)
                        # rank-1 mean fold: += w1[chan] * (mu*rstd)[token]
                        w1sl = w1_sb[0:1, (m % 2) * 128 + (m // 2) * DL:
                                     (m % 2) * 128 + (m // 2) * DL + 128]
                        nc.tensor.matmul(ps[:], w1sl,
                                         mursr[0:1, ch * 512:(ch + 1) * 512],
                                         start=False, stop=True)
                        b, col = ch // 4, (ch % 4) * 512
                        if m < 2:
                            dst = xpad[m, b][:, 3 + col:3 + col + 512]
                            nc.scalar.activation(dst, ps[:], AF.Identity,
                                                 bias=bx_sb[m][:])
                        else:
                            dst = zs16[m - 2][:, ch * 512:ch * 512 + 512]
                            nc.scalar.activation(dst, ps[:], AF.Silu,
                                                 bias=bz_sb[m - 2][:])

                def conv_dir(hps, dr, xrev):
                    for m in range(2):
                        for b in range(2):
                            src_t = xpad[m, b] if dr == 0 else xrev[m, b]
                            for c in range(4):
                                ps = hps.tile([128, 512], f32, tag="ps", name="ps")
                                for j in range(KCV):
                                    rhs = src_t[:, j + c * 512:j + c * 512 + 512]
                                    nc.tensor.matmul(ps[:], cvd_sb[dr, j, m], rhs,
                                                     start=(j == 0), stop=(j == KCV - 1))
                                dst = u16[dr, m][:, b * L + c * 512:b * L + (c + 1) * 512]
                                nc.scalar.activation(dst, ps[:], AF.Silu,
                                                     bias=cb_sb[dr, m][:])

                def xproj_dir(hps, cwk, dr):
                    for ch in range(8):
                        ps = hps.tile([NXP, 512], f32, tag="ps2", name="ps2", bufs=2)
                        for m in range(2):
                            nc.tensor.matmul(ps[:], xw_sb[dr, m],
                                             u16[dr, m][:, ch * 512:(ch + 1) * 512],
                                             start=(m == 0), stop=(m == 1))
                        xc = cwk.tile([NXP, 512], f16, tag="xc", name="xc", bufs=3)
                        nc.scalar.activation(xc[:], ps[:], AF.Identity)
                        nc.sync.dma_start(
                            ar_src[dr][:, ch * 512:(ch + 1) * 512], xc[:])
                    nc.gpsimd.collective_compute(
                        "AllReduce", AL.add, replica_groups=[list(range(NC_))],
                        ins=[ar_src[dr].opt()], outs=[ar_dst[dr].opt()])

                # dt chain: dt = softplus(dtw @ dtpart + dtb), dtu, poison col
                def make_dt(dr, m, pool):
                    dt_ = pool.tile([128, TOK], f16, tag=f"dt{m}", name=f"dt{dr}{m}")
                    du_ = pool.tile([128, TOK], f16, tag=f"du{m}", name=f"du{dr}{m}")
                    with tc.tile_pool(name=f"dps{dr}{m}", bufs=2, space="PSUM") as dps, \
                         tc.tile_pool(name=f"dwk{dr}{m}", bufs=1) as dwk:
                        dtp16 = dwk.tile([DTR, TOK], f16, tag="dtp16", name="dtp16")
                        nc.sync.dma_start(dtp16[:], ar_dst[dr][0:DTR, :])
                        for ch in range(8):
                            ps = dps.tile([128, 512], f32, tag="psd", name="psd")
                            nc.tensor.matmul(ps[:],
                                             dtw_sb[dr][:, m * 128:(m + 1) * 128],
                                             dtp16[:, ch * 512:(ch + 1) * 512],
                                             start=True, stop=True)
                            et = dwk.tile([128, 512], f32, tag="et", name="et", bufs=2)
                            nc.scalar.activation(et[:], ps[:], AF.Exp,
                                                 bias=dtb_sb[dr, m][:])
                            nc.scalar.activation(
                                dt_[:, ch * 512:(ch + 1) * 512], et[:],
                                AF.Ln, bias=1.0)
                        nc.vector.tensor_tensor(du_[:], dt_[:],
                                                u16[dr, m][:], AL.mult)
                        # poison the b-boundary decay column (after dtu!)
                        nc.vector.memset(dt_[:, L:L + 1], POISON)
                    return dt_, du_

                cm_xr = tc.tile_pool(name="xrv", bufs=1)
                xrv = cm_xr.__enter__()
                with tc.tile_pool(name="hps", bufs=4, space="PSUM") as hps, \
                     tc.tile_pool(name="cwk", bufs=1) as cwk:
                    if _rep == 0:
                        for _t, _s in wload[20:]:   # conv/scan weights
                            nc.sync.dma_start(_t, _s)
                    in_proj_half(hps, 0)
                    in_proj_half(hps, 1)
                    xrev = {}
                    for m in range(2):
                        for b in range(2):
                            t = xrv.tile([128, L + 6], f16, tag=f"xr{m}{b}", name=f"xr{m}{b}")
                            nc.vector.tensor_copy(t[:], xpad[m, b][:, L + 5::-1])
                            xrev[m, b] = t
                    conv_dir(hps, 0, xrev)
                    xproj_dir(hps, cwk, 0)
                    # z-half m2 + conv1 cover the AR0 latency on PE, then the
                    # dir-0/m0 dt chain runs immediately so the first scan
                    # starts ASAP; z-m3/xproj1/AR1/dt01 hide under the scans
                    in_proj_half(hps, 2)
                    conv_dir(hps, 1, xrev)
                    dt00, du00 = make_dt(0, 0, dtA)
                    in_proj_half(hps, 3)
                    xproj_dir(hps, cwk, 1)
                cm_xr.__exit__(None, None, None)    # free xrev
                cm_rhn.__exit__(None, None, None)   # free hn (+mursr)
                cm_xp.__exit__(None, None, None)    # free xpad

                # ======== scan blocks: (dr, m), merged-b [128, 4096] ========
                a2a_src = [dram.tile([NC_ * 128, TSL], f16, tag=f"a2s{m}", name=f"a2s{m}")
                           for m in range(2)]
                a2a_dst = [dram.tile([NC_ * 128, TSL], f16, tag=f"a2d{m}", name=f"a2d{m}")
                           for m in range(2)]

                REVC = [3, 2, 1, 0, 7, 6, 5, 4]   # per-b chunk reversal map

                def emit_block(dr, m, dt_, du_, bcp, hwp):
                    with tc.tile_pool(name=f"eps{dr}{m}", bufs=1, space="PSUM") as eps:
                        py = [eps.tile([128, 512], f32, tag=f"py{c}", name=f"py{c}")
                              for c in range(8)]
                        for n in range(NST):
                            bt = bcp.tile([128, TOK], f16, tag="bt", name="bt",
                                          bufs=1)
                            nc.sync.dma_start(
                                bt[:], ar_dst[dr][DTR + n:DTR + n + 1,
                                                  :].broadcast_to((128, TOK)))
                            ct = bcp.tile([128, TOK], f16, tag="ct", name="ct")
                            nc.sync.dma_start(
                                ct[:], ar_dst[dr][DTR + NST + n:DTR + NST + n + 1,
                                                  :].broadcast_to((128, TOK)))
                            a16 = hwp.tile([128, TOK], f16, tag="a16", name="a16")
                            nc.scalar.activation(a16[:], dt_[:], AF.Exp,
                                                 scale=at_sb[dr, m][:, n:n + 1])
                            xs = hwp.tile([128, TOK], f16, tag="xs", name="xs")
                            nc.vector.tensor_tensor(xs[:], du_[:], bt[:], AL.mult)
                            if n in GP_N:
                                h16 = hwp.tile([128, TOK], f16, tag="h16g", name="h16g",
                                               bufs=1)
                                nc.vector.tensor_tensor_scan(h16[:], a16[:], xs[:],
                                                             0.0, AL.mult, AL.add)
                                nc.gpsimd.tensor_tensor(h16[:], h16[:], ct[:], AL.mult)
                            else:
                                h16 = hwp.tile([128, TOK], f16, tag="h16", name="h16")
                                nc.vector.tensor_tensor_scan(h16[:], a16[:], xs[:],
                                                             0.0, AL.mult, AL.add)
                                nc.vector.tensor_tensor(h16[:], h16[:], ct[:], AL.mult)
                            for c in range(8):
                                nc.tensor.matmul(py[c][:], i128_sb[:],
                                                 h16[:, c * 512:(c + 1) * 512],
                                                 start=(n == 0), stop=False)
                        for c in range(8):
                            nc.tensor.matmul(py[c][:], dpd_sb[dr, m],
                                             u16[dr, m][:, c * 512:(c + 1) * 512],
                                             start=False, stop=True)
                        for c in range(8):
                            csl = slice(c * 512, (c + 1) * 512)
                            if dr == 0:
                                nc.vector.tensor_tensor(y16[m][:, csl], py[c][:],
                                                        zs16[m][:, csl], AL.mult)
                            else:
                                gt = hwp.tile([128, 512], f16, tag="gt", name="gt",
                                              bufs=1)
                                rev = py[REVC[c]][:, 511::-1]
                                nc.vector.tensor_tensor(gt[:], rev,
                                                        zs16[m][:, csl], AL.mult)
                                nc.vector.tensor_tensor(y16[m][:, csl],
                                                        y16[m][:, csl], gt[:], AL.add)

                def a2a_write(m):
                    for j in range(NC_):
                        nc.sync.dma_start(
                            a2a_src[m][j * 128:(j + 1) * 128, :],
                            y16[m][:, j * TSL:(j + 1) * TSL])
                    nc.gpsimd.collective_compute(
                        "AllToAll", AL.bypass, replica_groups=[list(range(NC_))],
                        ins=[a2a_src[m].opt()], outs=[a2a_dst[m].opt()])

                with tc.tile_pool(name="bcp", bufs=2) as bcp, \
                     tc.tile_pool(name="hwp", bufs=2) as hwp, \
                     tc.tile_pool(name="dtB", bufs=1) as dtB:
                    y16[0] = hwp.tile([128, TOK], f16, tag="y0", name="y0", bufs=1)
                    y16[1] = hwp.tile([128, TOK], f16, tag="y1", name="y1", bufs=1)
                    emit_block(0, 0, dt00, du00, bcp, hwp)
                    dt01, du01 = make_dt(0, 1, dtB)      # one dt chain per gap
                    emit_block(0, 1, dt01, du01, bcp, hwp)
                    dt10, du10 = make_dt(1, 0, dtA)      # reuses (0,0) dt space
                    emit_block(1, 0, dt10, du10, bcp, hwp)
                    dt11, du11 = make_dt(1, 1, dtB)      # reuses (0,1) dt space
                    a2a_write(0)
                    emit_block(1, 1, dt11, du11, bcp, hwp)
                    a2a_write(1)
                cm_dtA.__exit__(None, None, None)
                cm_u.__exit__(None, None, None)
                cm_zs.__exit__(None, None, None)

                # ============ Phase G: out_proj on gathered y ===============
                with tc.tile_pool(name="gps", bufs=1, space="PSUM") as gps, \
                     tc.tile_pool(name="gwk", bufs=3) as gwk, \
                     tc.tile_pool(name="gya", bufs=1) as gya:
                    wop_sb = {}
                    for m in range(2):
                        for i in range(NC_):
                            t = gya.tile([128, D], f16, tag=f"wo{m}{i}", name=f"wo{m}{i}")
                            nc.sync.dma_start(
                                t[:], wop_in[i * DL + m * 128:i * DL + (m + 1) * 128, :])
                            wop_sb[m, i] = t
                    yall = {}
                    for m in range(2):
                        for i in range(NC_):
                            t = gya.tile([128, TSL], f16, tag=f"ya{m}{i}", name=f"ya{m}{i}")
                            nc.sync.dma_start(t[:], a2a_dst[m][i * 128:(i + 1) * 128, :])
                            yall[m, i] = t
                    ps = [gps.tile([128, TSL], f32, tag=f"ops{mt}", name=f"ops{mt}")
                          for mt in range(8)]
                    for mt in range(8):
                        for m in range(2):
                            for i in range(NC_):
                                nc.tensor.matmul(ps[mt][:],
                                                 wop_sb[m, i][:, mt * 128:(mt + 1) * 128],
                                                 yall[m, i][:],
                                                 start=(m == 0 and i == 0),
                                                 stop=(m == 1 and i == NC_ - 1))
                        o32 = gwk.tile([128, TSL], f32, tag="o32", name="o32")
                        nc.scalar.activation(o32[:], ps[mt][:], AF.Identity,
                                             bias=opb_sb[mt][:])
                        nc.sync.dma_start(o_out[mt * 128:(mt + 1) * 128, :], o32[:])
    return nc


# ----------------------------------------------------------------- host

def _host_prep(inputs):
    """Build per-core input dicts from the full-model inputs."""
    gam = np.asarray(inputs["gamma"], np.float32)
    bet = np.asarray(inputs["beta"], np.float32)
    wip = np.asarray(inputs["in_proj_w"], np.float32)     # (2*DIN, D)
    wop = np.asarray(inputs["out_proj_w"], np.float32)    # (D, DIN)
    opb = np.asarray(inputs["out_proj_b"], np.float32)
    hs = np.asarray(inputs["hidden_states"], np.float32)
    res = np.asarray(inputs["residual"], np.float32)

    conv_w = [np.asarray(inputs["conv_w"], np.float32),
              np.asarray(inputs["conv_w_b"], np.float32)]
    conv_b = [np.asarray(inputs["conv_b"], np.float32),
              np.asarray(inputs["conv_b_b"], np.float32)]
    xw = [np.asarray(inputs["xproj_w"], np.float32),
          np.asarray(inputs["xproj_w_b"], np.float32)]
    dtw = [np.asarray(inputs["dtproj_w"], np.float32),
           np.asarray(inputs["dtproj_w_b"], np.float32)]
    dtb = [np.asarray(inputs["dtproj_b"], np.float32),
           np.asarray(inputs["dtproj_b_b"], np.float32)]
    alog = [np.asarray(inputs["A_log"], np.float32),
            np.asarray(inputs["A_b_log"], np.float32)]
    dp = [np.asarray(inputs["Dp"], np.float32),
          np.asarray(inputs["Dp_b"], np.float32)]

    wip_g = wip * gam[None, :]           # fold gamma
    bias_full = wip @ bet                # fold beta  (2*DIN,)

    i128 = np.eye(128, dtype=np.float16)
    ones = np.ones((128, 1), np.float16)

    # token-major flattening of hs/res: (B, D, L) -> (D, B*L)
    hs_f = hs.transpose(1, 0, 2).reshape(D, TOK)
    res_f = res.transpose(1, 0, 2).reshape(D, TOK)
    hs16 = hs_f.astype(np.float16)
    res16 = res_f.astype(np.float16)

    in_maps = []
    for i in range(NC_):
        ds = slice(i * DL, (i + 1) * DL)
        wxT = wip_g[ds, :].T.astype(np.float16)               # (D, DL)
        wzT = wip_g[DIN + i * DL:DIN + (i + 1) * DL, :].T.astype(np.float16)
        w1s = np.concatenate([
            -wip_g[ds, :].sum(axis=1),
            -wip_g[DIN + i * DL:DIN + (i + 1) * DL, :].sum(axis=1),
        ]).reshape(1, 2 * DL).astype(np.float16)
        bx = bias_full[ds].reshape(DL, 1).astype(np.float32)
        bz = bias_full[DIN + i * DL:DIN + (i + 1) * DL].reshape(DL, 1).astype(np.float32)
        cvd = np.zeros((2, KCV, 2, 128, 128), np.float16)
        cb = np.zeros((2, DL, 1), np.float32)
        xwT = np.zeros((2, DL, DTR + 2 * NST), np.float16)
        dtwT = np.zeros((2, DTR, DL), np.float16)
        dtbv = np.zeros((2, DL, 1), np.float32)
        atab = np.zeros((2, DL, NST), np.float32)
        dpd = np.zeros((2, 2, 128, 128), np.float16)
        for dr in range(2):
            w = conv_w[dr][ds, 0, :]                          # (DL, KCV)
            for j in range(KCV):
                for m in range(2):
                    cvd[dr, j, m] = np.diag(w[m * 128:(m + 1) * 128, j]).astype(np.float16)
            cb[dr] = conv_b[dr][ds].reshape(DL, 1)
            xwT[dr] = xw[dr][:, ds].T.astype(np.float16)      # (DL, 96)
            dtwT[dr] = dtw[dr][ds, :].T.astype(np.float16)    # (DTR, DL)
            dtbv[dr] = dtb[dr][ds].reshape(DL, 1)
            atab[dr] = -np.exp(alog[dr][ds, :])
            for m in range(2):
                dpd[dr, m] = np.diag(dp[dr][ds][m * 128:(m + 1) * 128]).astype(np.float16)
        in_maps.append({
            "hsf": hs16, "resf": res16,
            "hss": np.ascontiguousarray(hs_f[:, i * TSL:(i + 1) * TSL]),
            "ress": np.ascontiguousarray(res_f[:, i * TSL:(i + 1) * TSL]),
            "wxT": wxT, "wzT": wzT, "w1s": w1s, "bx": bx, "bz": bz,
            "convdiag": cvd, "convb": cb,
            "xwT": xwT, "dtwT": dtwT, "dtb": dtbv, "atab": atab,
            "dpdiag": dpd,
            "wopT": wop.T.astype(np.float16),                 # (DIN, D)
            "opb": opb.reshape(D, 1).astype(np.float32),
            "i128": i128, "ones": ones,
        })
    return in_maps


class _Exec:
    """Compile once; run via PJRT shard_map on 8 cores."""

    def __init__(self, nc, n_cores):
        from jax.sharding import Mesh, PartitionSpec
        from jax.experimental.shard_map import shard_map
        bass2jax.install_neuronx_cc_hook()
        self.nc = nc
        self.n = n_cores
        partition_name = nc.partition_id_tensor.name if nc.partition_id_tensor else None
        in_names, out_names, out_avals, zero_outs = [], [], [], []
        for alloc in nc.m.functions[0].allocations:
            if not isinstance(alloc, mybir.MemoryLocationSet):
                continue
            name = alloc.memorylocations[0].name
            if alloc.kind == "ExternalInput":
                if name != partition_name:
                    in_names.append(name)
            elif alloc.kind == "ExternalOutput":
                shape = tuple(alloc.tensor_shape)
                npdt = mybir.dt.np(alloc.dtype)
                out_names.append(name)
                out_avals.append(jax.core.ShapedArray(shape, npdt))
                zero_outs.append(np.zeros(shape, npdt))
        self.in_names, self.out_names = in_names, out_names
        self.out_avals, self.zero_outs = out_avals, zero_outs
        all_in = list(in_names) + list(out_names)
        if partition_name is not None:
            all_in.append(partition_name)

        def _body(*args):
            operands = list(args)
            if partition_name is not None:
                operands.append(bass2jax.partition_id_tensor())
            outs = bass2jax._bass_exec_p.bind(
                *operands,
                out_avals=tuple(out_avals),
                in_names=tuple(all_in),
                out_names=tuple(out_names),
                lowering_input_output_aliases=(),
                sim_require_finite=True,
                sim_require_nnan=True,
                nc=nc,
            )
            return tuple(outs)

        devices = jax.devices()[:n_cores]
        self.mesh = Mesh(np.asarray(devices), ("core",))
        np_ = len(in_names) + len(out_names)
        self.fn = jax.jit(
            shard_map(_body, mesh=self.mesh,
                      in_specs=(PartitionSpec("core"),) * np_,
                      out_specs=(PartitionSpec("core"),) * len(out_names),
                      check_rep=False),
            keep_unused=True)

    def prep(self, in_maps):
        from jax.sharding import NamedSharding, PartitionSpec
        n = self.n
        cat = [np.concatenate([np.asarray(in_maps[c][k]) for c in range(n)], axis=0)
               for k in self.in_names]
        cat += [np.zeros((n * z.shape[0], *z.shape[1:]), z.dtype)
                for z in self.zero_outs]
        sh = NamedSharding(self.mesh, PartitionSpec("core"))
        return [jax.device_put(a, sh) for a in cat]

    def run(self, args):
        outs = self.fn(*args)
        jax.block_until_ready(outs)
        return outs

    def results(self, outs):
        n = self.n
        return [
            {name: np.asarray(outs[i]).reshape(n, *self.out_avals[i].shape)[c]
             for i, name in enumerate(self.out_names)}
            for c in range(n)
        ]


_EXEC = None


def _get_exec():
    global _EXEC
    if _EXEC is None:
        _EXEC = _Exec(build_program(), NC_)
    return _EXEC


def kernel(**inputs):
    e = _get_exec()
    in_maps = _host_prep(inputs)
    res = e.results(e.run(e.prep(in_maps)))
    out = np.zeros((B, D, L), np.float32)
    r = np.zeros((B, D, L), np.float32)
    for i in range(NC_):
        b = i // 4
        l0 = (i % 4) * TSL
        out[b][:, l0:l0 + TSL] = res[i]["o_out"]
        r[b][:, l0:l0 + TSL] = res[i]["r_out"]
    return out, r


# revision 48
# speedup vs baseline: 1.0188x; 1.0188x over previous
"""Bimamba (bidirectional Mamba) block on 8 trn2 NeuronCores.

Sharding: tensor-parallel over d_inner (256 channels/core). LayerNorm is
computed redundantly per core on full-token f16 inputs (no AllGather);
x_proj partial sums are AllReduced per direction in f16; out_proj is
resolved with two m-split token AllToAlls so the first half overlaps the
final scan block. The selective scan runs b-merged [128, 4096] via a
poison-column decay reset at the batch boundary.
"""
import sys, os, json, time

sys.path.insert(0, '/opt/trn_rl_repo')

import numpy as np
import concourse.bass as bass
import concourse.mybir as mybir
import concourse.tile as tile
import bass_rust
from concourse.vector_clock import ScopedClock
from concourse import bass2jax
import jax

# ----------------------------------------------------------------- patches

def _patched_drain_and_barrier(self, tick_clock, wait_clock):
    nc = self.nc
    gc = tick_clock.global_clock
    vals = json.loads(repr(gc).replace("VectorClock(", "").rstrip(")"))
    procs = [i for i, v in enumerate(vals) if v > 0]
    for p in procs:
        sub = bass_rust.VectorClock()
        sub.require_at_least(p, vals[p])
        nop = nc.sync.nop(nofuse=True)
        wait_clock.add_sem_waits(nop.ins, ScopedClock({None: sub}))
    nc.sync.drain()
    nc.all_engine_barrier()
    assert self.sems is not None
    popped = nc._tile_sem_poison_stack.pop()
    assert popped is self._sem_poison
    nc.clear_and_free_semaphores(list(self.sems.allocated().values()))
    nc.all_engine_barrier()


tile.TileContext._drain_and_barrier = _patched_drain_and_barrier

_SPLIT_ENGINES = {"SP", "PE", "DVE", "Activation", "Pool"}
_wsplit_ctr = [0]


def _split_excess_waits(bir, max_waits=1):
    for f in bir.get("functions") or []:
        for blk in f.get("blocks") or []:
            insts = blk.get("instructions") or []
            out = []
            for inst in insts:
                si = inst.get("sync_info")
                waits = (si or {}).get("on_wait") or []
                eng = inst.get("engine")
                if len(waits) > max_waits and eng in _SPLIT_ENGINES:
                    keep, extra = waits[:max_waits], waits[max_waits:]
                    for i in range(0, len(extra), max_waits):
                        _wsplit_ctr[0] += 1
                        out.append({
                            "debug": inst.get("debug", 0),
                            "engine": eng,
                            "ins": [], "outs": [],
                            "name": f"WSPLIT-{_wsplit_ctr[0]}",
                            "opcode": "NoOp",
                            "sync_info": {"on_update": [],
                                          "on_wait": extra[i:i + max_waits]},
                        })
                    si["on_wait"] = keep
                out.append(inst)
            blk["instructions"] = out
    return bir


if not getattr(bass.Bass, "_ws_patched", False):
    _orig_to_json_bytes = bass.Bass.to_json_bytes

    def _patched_to_json_bytes(self):
        bir = json.loads(_orig_to_json_bytes(self))
        _split_excess_waits(bir)
        return json.dumps(bir).encode()

    bass.Bass.to_json_bytes = _patched_to_json_bytes
    bass.Bass._ws_patched = True

# ----------------------------------------------------------------- consts

B, D, L = 2, 1024, 2048
DIN, NST, DTR, KCV = 2048, 16, 64, 4
NC_ = 8
DL = DIN // NC_          # 256 channels per core
TOK = B * L              # 4096 tokens, b-major
TSL = TOK // NC_         # 512-token slice per core
EPS = 1e-5

f32 = mybir.dt.float32
f16 = mybir.dt.float16
AL = mybir.AluOpType
AF = mybir.ActivationFunctionType

NXP = DTR + 2 * NST      # 96
POISON = 60000.0         # dt poison at the b-boundary column: exp(A*POISON)=0

# which n-iterations route their C-mult to GpSimd (per scan block).
# Empirically net-negative on trn2: POOL shares an SBUF port with DVE and
# concurrent gp tensor_tensor slows the DVE scans ~15%. Keep empty.
GP_N = ()


# ----------------------------------------------------------------- program

def build_program(reps=1):
    nc = bass.Bass(trn_type="TRN2", target_bir_lowering=False, num_devices=NC_)

    def din(name, shape, dt=f32):
        return nc.dram_tensor(name, list(shape), dt, kind="ExternalInput").ap()

    def dout(name, shape, dt=f32):
        return nc.dram_tensor(name, list(shape), dt, kind="ExternalOutput").ap()

    hsf_in = din("hsf", (D, TOK), f16)       # full tokens, f16 (LN is redundant)
    resf_in = din("resf", (D, TOK), f16)
    hss_in = din("hss", (D, TSL))            # per-core f32 slices (exact r_out)
    ress_in = din("ress", (D, TSL))
    wx_in = din("wxT", (D, DL), f16)         # in_proj x-rows lhsT (gamma folded)
    wz_in = din("wzT", (D, DL), f16)
    w1_in = din("w1s", (1, 2 * DL), f16)     # -(row sums) for the mu rank-1 fold
    bx_in = din("bx", (DL, 1))               # in_proj beta-fold biases
    bz_in = din("bz", (DL, 1))
    cvd_in = din("convdiag", (2, KCV, 2, 128, 128), f16)   # (dir,tap,m,.,.)
    cb_in = din("convb", (2, DL, 1))
    xw_in = din("xwT", (2, DL, NXP), f16)    # (dir, k=dl, 96)
    dtw_in = din("dtwT", (2, DTR, DL), f16)
    dtb_in = din("dtb", (2, DL, 1))
    atab_in = din("atab", (2, DL, NST))
    dpd_in = din("dpdiag", (2, 2, 128, 128), f16)
    wop_in = din("wopT", (DIN, D), f16)
    opb_in = din("opb", (D, 1))
    i128_in = din("i128", (128, 128), f16)
    ones_in = din("ones", (128, 1), f16)

    r_out = dout("r_out", (D, TSL))          # per-core r token slice
    o_out = dout("o_out", (D, TSL))          # out token-slice (per core)

    with tile.TileContext(nc) as tc:
        with tc.tile_pool(name="wts", bufs=1) as wts, \
             tc.tile_pool(name="dram", bufs=1, space="DRAM") as dram:

            # ---- small weights; DMAs deferred until after LN input loads
            wload = []

            def wdma(t, s):
                wload.append((t, s))

            wx_sb = [wts.tile([128, DL], f16, tag=f"wx{k}", name=f"wx{k}") for k in range(8)]
            wz_sb = [wts.tile([128, DL], f16, tag=f"wz{k}", name=f"wz{k}") for k in range(8)]
            for k in range(8):
                wdma(wx_sb[k][:], wx_in[k * 128:(k + 1) * 128, :])
                wdma(wz_sb[k][:], wz_in[k * 128:(k + 1) * 128, :])
            bx_sb = [wts.tile([128, 1], f32, tag=f"bx{m}", name=f"bx{m}") for m in range(2)]
            bz_sb = [wts.tile([128, 1], f32, tag=f"bz{m}", name=f"bz{m}") for m in range(2)]
            for m in range(2):
                wdma(bx_sb[m][:], bx_in[m * 128:(m + 1) * 128, :])
                wdma(bz_sb[m][:], bz_in[m * 128:(m + 1) * 128, :])
            cvd_sb = {}
            for dr in range(2):
                for j in range(KCV):
                    for m in range(2):
                        t = wts.tile([128, 128], f16, tag=f"cv{dr}{j}{m}", name=f"cv{dr}{j}{m}")
                        wdma(t[:], cvd_in[dr, j, m])
                        cvd_sb[dr, j, m] = t
            cb_sb = {}
            dtb_sb = {}
            at_sb = {}
            dpd_sb = {}
            for dr in range(2):
                for m in range(2):
                    t = wts.tile([128, 1], f32, tag=f"cb{dr}{m}", name=f"cb{dr}{m}")
                    wdma(t[:], cb_in[dr, m * 128:(m + 1) * 128, :])
                    cb_sb[dr, m] = t
                    t = wts.tile([128, 1], f32, tag=f"db{dr}{m}", name=f"db{dr}{m}")
                    wdma(t[:], dtb_in[dr, m * 128:(m + 1) * 128, :])
                    dtb_sb[dr, m] = t
                    t = wts.tile([128, NST], f32, tag=f"at{dr}{m}", name=f"at{dr}{m}")
                    wdma(t[:], atab_in[dr, m * 128:(m + 1) * 128, :])
                    at_sb[dr, m] = t
                    t = wts.tile([128, 128], f16, tag=f"dp{dr}{m}", name=f"dp{dr}{m}")
                    wdma(t[:], dpd_in[dr, m])
                    dpd_sb[dr, m] = t
            xw_sb = {}
            for dr in range(2):
                for m in range(2):
                    t = wts.tile([128, NXP], f16, tag=f"xw{dr}{m}", name=f"xw{dr}{m}")
                    wdma(t[:], xw_in[dr, m * 128:(m + 1) * 128, :])
                    xw_sb[dr, m] = t
            dtw_sb = {}
            for dr in range(2):
                t = wts.tile([DTR, DL], f16, tag=f"dtw{dr}", name=f"dtw{dr}")
                wdma(t[:], dtw_in[dr])
                dtw_sb[dr] = t
            i128_sb = wts.tile([128, 128], f16, tag="i128", name="i128")
            wdma(i128_sb[:], i128_in)
            w1_sb = wts.tile([1, 2 * DL], f16, tag="w1s", name="w1s")
            wdma(w1_sb[:], w1_in)
            ones_sb = wts.tile([128, 1], f16, tag="ones", name="ones")
            nc.sync.dma_start(ones_sb[:], ones_in)
            opb_sb = [wts.tile([128, 1], f32, tag=f"opb{m}", name=f"opb{m}") for m in range(8)]
            for m in range(8):
                wdma(opb_sb[m][:], opb_in[m * 128:(m + 1) * 128, :])

            for _rep in range(reps):
                cm_zs = tc.tile_pool(name="zsp", bufs=1)
                zsp = cm_zs.__enter__()
                zs16 = [zsp.tile([128, TOK], f16, tag=f"zs{m}", name=f"zs{m}") for m in range(2)]
                cm_u = tc.tile_pool(name="up", bufs=1)
                upool = cm_u.__enter__()
                u16 = {(dr, m): upool.tile([128, TOK], f16, tag=f"u{dr}{m}", name=f"u{dr}{m}")
                       for dr in range(2) for m in range(2)}
                cm_dtA = tc.tile_pool(name="dtA", bufs=1)
                dtA = cm_dtA.__enter__()
                y16 = [None, None]   # allocated in the scan-block pool
                cm_xp = tc.tile_pool(name="xpp", bufs=1)
                xpp = cm_xp.__enter__()
                xpad = {(m, b): xpp.tile([128, L + 6], f16, tag=f"xp{m}{b}", name=f"xp{m}{b}")
                        for m in range(2) for b in range(2)}
                for m in range(2):
                    for b in range(2):
                        nc.vector.memset(xpad[m, b][:, 0:3], 0.0)
                        nc.vector.memset(xpad[m, b][:, L + 3:L + 6], 0.0)

                # ======== Phase A: redundant LN over all 4096 tokens =========
                # hn[k] holds r in f16, later normalized in place.
                cm_rhn = tc.tile_pool(name="rhn", bufs=1)
                rhn = cm_rhn.__enter__()
                hn = [rhn.tile([128, TOK], f16, tag=f"hn{k}", name=f"hn{k}")
                      for k in range(8)]
                with tc.tile_pool(name="lnw", bufs=1) as lnw, \
                     tc.tile_pool(name="lnps", bufs=1, space="PSUM") as lnps, \
                     tc.tile_pool(name="lnsm", bufs=1) as lnsm:
                    sst = [lnps.tile([33, 512], f32, tag=f"ss{c}", name=f"ss{c}")
                           for c in range(8)]
                    for k in range(8):
                        nc.sync.dma_start(hn[k][:], hsf_in[k * 128:(k + 1) * 128, :])
                        for hh in range(2):
                            hsl = slice(hh * L, (hh + 1) * L)
                            re_t = lnw.tile([128, L], f16, tag="re", name="re_t", bufs=2)
                            nc.sync.dma_start(
                                re_t[:], resf_in[k * 128:(k + 1) * 128, hsl])
                            nc.vector.tensor_tensor(hn[k][:, hsl], hn[k][:, hsl],
                                                    re_t[:], AL.add)
                            sq_t = lnw.tile([128, L], f16, tag="sqt", name="sq_t", bufs=1)
                            nc.scalar.activation(sq_t[:], hn[k][:, hsl], AF.Square)
                            for cc in range(4):
                                c = hh * 4 + cc
                                csl = slice(c * 512, (c + 1) * 512)
                                qsl = slice(cc * 512, (cc + 1) * 512)
                                nc.tensor.matmul(sst[c][0:1, :], ones_sb[:],
                                                 hn[k][:, csl],
                                                 start=(k == 0), stop=(k == 7))
                                nc.tensor.matmul(sst[c][32:33, :], ones_sb[:],
                                                 sq_t[:, qsl],
                                                 start=(k == 0), stop=(k == 7))
                    # exact r slice for r_out (f32 path, off critical path)
                    for k in range(8):
                        hs_s = lnw.tile([128, TSL], f32, tag="hss", name="hs_s", bufs=1)
                        re_s = lnw.tile([128, TSL], f32, tag="res", name="re_s", bufs=1)
                        nc.sync.dma_start(hs_s[:], hss_in[k * 128:(k + 1) * 128, :])
                        nc.sync.dma_start(re_s[:], ress_in[k * 128:(k + 1) * 128, :])
                        nc.vector.tensor_tensor(hs_s[:], hs_s[:], re_s[:], AL.add)
                        nc.sync.dma_start(r_out[k * 128:(k + 1) * 128, :], hs_s[:])
                    # stats rows gathered to [8, 512] (partition-parallel recip)
                    mu8 = lnsm.tile([8, 512], f16, tag="mu8", name="mu8")
                    ex8 = lnsm.tile([8, 512], f16, tag="ex8", name="ex8")
                    for c in range(8):
                        s1 = lnsm.tile([1, 512], f16, tag="s1", name="s1", bufs=2)
                        s2 = lnsm.tile([1, 512], f16, tag="s2", name="s2", bufs=2)
                        nc.vector.tensor_scalar_mul(s1[:], sst[c][0:1, :], 1.0 / D)
                        nc.vector.tensor_scalar_mul(s2[:], sst[c][32:33, :], 1.0 / D)
                        nc.sync.dma_start(mu8[c:c + 1, :], s1[:])
                        nc.sync.dma_start(ex8[c:c + 1, :], s2[:])
                    tmp8 = lnsm.tile([8, 512], f16, tag="tmp8", name="tmp8")
                    nc.vector.tensor_tensor(tmp8[:], mu8[:], mu8[:], AL.mult)
                    nc.vector.tensor_tensor(ex8[:], ex8[:], tmp8[:], AL.subtract)
                    nc.vector.tensor_scalar_add(ex8[:], ex8[:], float(EPS))
                    nc.scalar.activation(ex8[:], ex8[:], AF.Sqrt)
                    with nc.allow_low_precision(reason="LN stats in f16; tol 2e-2"):
                        nc.vector.reciprocal(tmp8[:], ex8[:])
                    r16_ = lnsm.tile([8, 512], f16, tag="r16_", name="r16_")
                    m16_ = lnsm.tile([8, 512], f16, tag="m16_", name="m16_")
                    nc.vector.tensor_copy(r16_[:], tmp8[:])
                    # murs = mu * rstd (folded into in_proj as a rank-1 term)
                    nc.vector.tensor_tensor(m16_[:], mu8[:], tmp8[:], AL.mult)
                    drow = dram.tile([2, TOK], f16, tag="stat", name="stat")
                    for c in range(8):
                        csl = slice(c * 512, (c + 1) * 512)
                        nc.sync.dma_start(drow[0:1, csl], r16_[c:c + 1, :])
                        nc.sync.dma_start(drow[1:2, csl], m16_[c:c + 1, :])
                    mursr = rhn.tile([1, TOK], f16, tag="mursr", name="mursr")
                    nc.sync.dma_start(mursr[:], drow[1:2, :])
                    for hh in range(2):
                        hsl = slice(hh * L, (hh + 1) * L)
                        rb = lnsm.tile([128, L], f16, tag="rbc", name="rbc", bufs=2)
                        nc.sync.dma_start(rb[:], drow[0:1, hsl].broadcast_to((128, L)))
                        for k in range(8):
                            nc.vector.tensor_tensor(hn[k][:, hsl], hn[k][:, hsl],
                                                    rb[:], AL.mult)

                if _rep == 0:
                    for _t, _s in wload[:20]:   # wx/wz/bx/bz: in_proj needs these
                        nc.sync.dma_start(_t, _s)

                # ======== Phase B/C: in_proj-x, conv0, xproj0, AR0, z, ... ===
                ar_src = [dram.tile([NXP, TOK], f16, tag=f"ars{dr}", name=f"ars{dr}")
                          for dr in range(2)]
                ar_dst = [dram.tile([NXP, TOK], f16, tag=f"ard{dr}", name=f"ard{dr}",
                                    addr_space="Shared") for dr in range(2)]

                def in_proj_half(hps, m):
                    # m in 0..3: 0,1 = x halves; 2,3 = z halves
                    for ch in range(8):
                        ps = hps.tile([128, 512], f32, tag="ps", name="ps")
                        for k in range(8):
                            w = wx_sb[k] if m < 2 else wz_sb[k]
                            lh = w[:, (m % 2) * 128:(m % 2) * 128 + 128]
                            nc.tensor.matmul(ps[:], lh,
                                             hn[k][:, ch * 512:(ch + 1) * 512],
                                             start=(k == 0), stop=False)
                        # rank-1 mean fold: += w1[chan] * (mu*rstd)[token]
                        w1sl = w1_sb[0:1, (m % 2) * 128 + (m // 2) * DL:
                                     (m % 2) * 128 + (m // 2) * DL + 128]
                        nc.tensor.matmul(ps[:], w1sl,
                                         mursr[0:1, ch * 512:(ch + 1) * 512],
                                         start=False, stop=True)
                        b, col = ch // 4, (ch % 4) * 512
                        if m < 2:
                            dst = xpad[m, b][:, 3 + col:3 + col + 512]
                            nc.scalar.activation(dst, ps[:], AF.Identity,
                                                 bias=bx_sb[m][:])
                        else:
                            dst = zs16[m - 2][:, ch * 512:ch * 512 + 512]
                            nc.scalar.activation(dst, ps[:], AF.Silu,
                                                 bias=bz_sb[m - 2][:])

                def conv_dir(hps, dr, xrev):
                    for m in range(2):
                        for b in range(2):
                            src_t = xpad[m, b] if dr == 0 else xrev[m, b]
                            for c in range(4):
                                ps = hps.tile([128, 512], f32, tag="ps", name="ps")
                                for j in range(KCV):
                                    rhs = src_t[:, j + c * 512:j + c * 512 + 512]
                                    nc.tensor.matmul(ps[:], cvd_sb[dr, j, m], rhs,
                                                     start=(j == 0), stop=(j == KCV - 1))
                                dst = u16[dr, m][:, b * L + c * 512:b * L + (c + 1) * 512]
                                nc.scalar.activation(dst, ps[:], AF.Silu,
                                                     bias=cb_sb[dr, m][:])

                def xproj_dir(hps, cwk, dr):
                    for ch in range(8):
                        ps = hps.tile([NXP, 512], f32, tag="ps2", name="ps2", bufs=2)
                        for m in range(2):
                            nc.tensor.matmul(ps[:], xw_sb[dr, m],
                                             u16[dr, m][:, ch * 512:(ch + 1) * 512],
                                             start=(m == 0), stop=(m == 1))
                        xc = cwk.tile([NXP, 512], f16, tag="xc", name="xc", bufs=3)
                        nc.scalar.activation(xc[:], ps[:], AF.Identity)
                        nc.sync.dma_start(
                            ar_src[dr][:, ch * 512:(ch + 1) * 512], xc[:])
                    nc.gpsimd.collective_compute(
                        "AllReduce", AL.add, replica_groups=[list(range(NC_))],
                        ins=[ar_src[dr].opt()], outs=[ar_dst[dr].opt()])

                # dt chain: dt = softplus(dtw @ dtpart + dtb), dtu, poison col
                def make_dt(dr, m, pool):
                    dt_ = pool.tile([128, TOK], f16, tag=f"dt{m}", name=f"dt{dr}{m}")
                    du_ = pool.tile([128, TOK], f16, tag=f"du{m}", name=f"du{dr}{m}")
                    with tc.tile_pool(name=f"dps{dr}{m}", bufs=2, space="PSUM") as dps, \
                         tc.tile_pool(name=f"dwk{dr}{m}", bufs=1) as dwk:
                        dtp16 = dwk.tile([DTR, TOK], f16, tag="dtp16", name="dtp16")
                        nc.sync.dma_start(dtp16[:], ar_dst[dr][0:DTR, :])
                        for ch in range(8):
                            ps = dps.tile([128, 512], f32, tag="psd", name="psd")
                            nc.tensor.matmul(ps[:],
                                             dtw_sb[dr][:, m * 128:(m + 1) * 128],
                                             dtp16[:, ch * 512:(ch + 1) * 512],
                                             start=True, stop=True)
                            et = dwk.tile([128, 512], f32, tag="et", name="et", bufs=2)
                            nc.scalar.activation(et[:], ps[:], AF.Exp,
                                                 bias=dtb_sb[dr, m][:])
                            nc.scalar.activation(
                                dt_[:, ch * 512:(ch + 1) * 512], et[:],
                                AF.Ln, bias=1.0)
                        nc.vector.tensor_tensor(du_[:], dt_[:],
                                                u16[dr, m][:], AL.mult)
                        # poison the b-boundary decay column (after dtu!)
                        nc.vector.memset(dt_[:, L:L + 1], POISON)
                    return dt_, du_

                cm_xr = tc.tile_pool(name="xrv", bufs=1)
                xrv = cm_xr.__enter__()
                with tc.tile_pool(name="hps", bufs=4, space="PSUM") as hps, \
                     tc.tile_pool(name="cwk", bufs=1) as cwk:
                    if _rep == 0:
                        for _t, _s in wload[20:]:   # conv/scan weights
                            nc.sync.dma_start(_t, _s)
                    in_proj_half(hps, 0)
                    in_proj_half(hps, 1)
                    xrev = {}
                    for m in range(2):
                        for b in range(2):
                            t = xrv.tile([128, L + 6], f16, tag=f"xr{m}{b}", name=f"xr{m}{b}")
                            nc.vector.tensor_copy(t[:], xpad[m, b][:, L + 5::-1])
                            xrev[m, b] = t
                    conv_dir(hps, 0, xrev)
                    xproj_dir(hps, cwk, 0)
                    # z-half m2 + conv1 cover the AR0 latency on PE, then the
                    # dir-0/m0 dt chain runs immediately so the first scan
                    # starts ASAP; z-m3/xproj1/AR1/dt01 hide under the scans
                    in_proj_half(hps, 2)
                    dt00, du00 = make_dt(0, 0, dtA)
                    in_proj_half(hps, 3)
                    conv_dir(hps, 1, xrev)
                    xproj_dir(hps, cwk, 1)
                cm_xr.__exit__(None, None, None)    # free xrev
                cm_rhn.__exit__(None, None, None)   # free hn (+mursr)
                cm_xp.__exit__(None, None, None)    # free xpad

                # ======== scan blocks: (dr, m), merged-b [128, 4096] ========
                a2a_src = [dram.tile([NC_ * 128, TSL], f16, tag=f"a2s{m}", name=f"a2s{m}")
                           for m in range(2)]
                a2a_dst = [dram.tile([NC_ * 128, TSL], f16, tag=f"a2d{m}", name=f"a2d{m}")
                           for m in range(2)]

                REVC = [3, 2, 1, 0, 7, 6, 5, 4]   # per-b chunk reversal map

                def emit_block(dr, m, dt_, du_, bcp, hwp):
                    with tc.tile_pool(name=f"eps{dr}{m}", bufs=1, space="PSUM") as eps:
                        py = [eps.tile([128, 512], f32, tag=f"py{c}", name=f"py{c}")
                              for c in range(8)]
                        # Dp term first (accumulation is order-free) so the
                        # final gating only waits on the last n's accums
                        for c in range(8):
                            nc.tensor.matmul(py[c][:], dpd_sb[dr, m],
                                             u16[dr, m][:, c * 512:(c + 1) * 512],
                                             start=True, stop=False)
                        for n in range(NST):
                            bt = bcp.tile([128, TOK], f16, tag="bt", name="bt",
                                          bufs=1)
                            nc.sync.dma_start(
                                bt[:], ar_dst[dr][DTR + n:DTR + n + 1,
                                                  :].broadcast_to((128, TOK)))
                            ct = bcp.tile([128, TOK], f16, tag="ct", name="ct")
                            nc.sync.dma_start(
                                ct[:], ar_dst[dr][DTR + NST + n:DTR + NST + n + 1,
                                                  :].broadcast_to((128, TOK)))
                            a16 = hwp.tile([128, TOK], f16, tag="a16", name="a16")
                            nc.scalar.activation(a16[:], dt_[:], AF.Exp,
                                                 scale=at_sb[dr, m][:, n:n + 1])
                            xs = hwp.tile([128, TOK], f16, tag="xs", name="xs")
                            nc.vector.tensor_tensor(xs[:], du_[:], bt[:], AL.mult)
                            if n in GP_N:
                                h16 = hwp.tile([128, TOK], f16, tag="h16g", name="h16g",
                                               bufs=1)
                                nc.vector.tensor_tensor_scan(h16[:], a16[:], xs[:],
                                                             0.0, AL.mult, AL.add)
                                nc.gpsimd.tensor_tensor(h16[:], h16[:], ct[:], AL.mult)
                            else:
                                h16 = hwp.tile([128, TOK], f16, tag="h16", name="h16")
                                nc.vector.tensor_tensor_scan(h16[:], a16[:], xs[:],
                                                             0.0, AL.mult, AL.add)
                                nc.vector.tensor_tensor(h16[:], h16[:], ct[:], AL.mult)
                            for c in range(8):
                                nc.tensor.matmul(py[c][:], i128_sb[:],
                                                 h16[:, c * 512:(c + 1) * 512],
                                                 start=False, stop=(n == NST - 1))
                        for c in range(8):
                            csl = slice(c * 512, (c + 1) * 512)
                            if dr == 0:
                                nc.vector.tensor_tensor(y16[m][:, csl], py[c][:],
                                                        zs16[m][:, csl], AL.mult)
                            else:
                                gt = hwp.tile([128, 512], f16, tag="gt", name="gt",
                                              bufs=1)
                                rev = py[REVC[c]][:, 511::-1]
                                nc.vector.tensor_tensor(gt[:], rev,
                                                        zs16[m][:, csl], AL.mult)
                                nc.vector.tensor_tensor(y16[m][:, csl],
                                                        y16[m][:, csl], gt[:], AL.add)

                def a2a_write(m):
                    for j in range(NC_):
                        nc.sync.dma_start(
                            a2a_src[m][j * 128:(j + 1) * 128, :],
                            y16[m][:, j * TSL:(j + 1) * TSL])
                    nc.gpsimd.collective_compute(
                        "AllToAll", AL.bypass, replica_groups=[list(range(NC_))],
                        ins=[a2a_src[m].opt()], outs=[a2a_dst[m].opt()])

                with tc.tile_pool(name="bcp", bufs=2) as bcp, \
                     tc.tile_pool(name="hwp", bufs=2) as hwp, \
                     tc.tile_pool(name="dtB", bufs=1) as dtB:
                    y16[0] = hwp.tile([128, TOK], f16, tag="y0", name="y0", bufs=1)
                    y16[1] = hwp.tile([128, TOK], f16, tag="y1", name="y1", bufs=1)
                    dt01, du01 = make_dt(0, 1, dtB)
                    emit_block(0, 0, dt00, du00, bcp, hwp)
                    dt10, du10 = make_dt(1, 0, dtA)      # reuses (0,0) dt space
                    emit_block(0, 1, dt01, du01, bcp, hwp)
                    dt11, du11 = make_dt(1, 1, dtB)      # reuses (0,1) dt space
                    emit_block(1, 0, dt10, du10, bcp, hwp)
                    a2a_write(0)
                    emit_block(1, 1, dt11, du11, bcp, hwp)
                    a2a_write(1)
                cm_dtA.__exit__(None, None, None)
                cm_u.__exit__(None, None, None)
                cm_zs.__exit__(None, None, None)

                # ============ Phase G: out_proj on gathered y ===============
                with tc.tile_pool(name="gps", bufs=1, space="PSUM") as gps, \
                     tc.tile_pool(name="gwk", bufs=3) as gwk, \
                     tc.tile_pool(name="gya", bufs=1) as gya:
                    wop_sb = {}
                    for m in range(2):
                        for i in range(NC_):
                            t = gya.tile([128, D], f16, tag=f"wo{m}{i}", name=f"wo{m}{i}")
                            nc.sync.dma_start(
                                t[:], wop_in[i * DL + m * 128:i * DL + (m + 1) * 128, :])
                            wop_sb[m, i] = t
                    yall = {}
                    for m in range(2):
                        for i in range(NC_):
                            t = gya.tile([128, TSL], f16, tag=f"ya{m}{i}", name=f"ya{m}{i}")
                            nc.sync.dma_start(t[:], a2a_dst[m][i * 128:(i + 1) * 128, :])
                            yall[m, i] = t
                    ps = [gps.tile([128, TSL], f32, tag=f"ops{mt}", name=f"ops{mt}")
                          for mt in range(8)]
                    for mt in range(8):
                        for m in range(2):
                            for i in range(NC_):
                                nc.tensor.matmul(ps[mt][:],
                                                 wop_sb[m, i][:, mt * 128:(mt + 1) * 128],
                                                 yall[m, i][:],
                                                 start=(m == 0 and i == 0),
                                                 stop=(m == 1 and i == NC_ - 1))
                        o32 = gwk.tile([128, TSL], f32, tag="o32", name="o32")
                        nc.scalar.activation(o32[:], ps[mt][:], AF.Identity,
                                             bias=opb_sb[mt][:])
                        nc.sync.dma_start(o_out[mt * 128:(mt + 1) * 128, :], o32[:])
    return nc


# ----------------------------------------------------------------- host

def _host_prep(inputs):
    """Build per-core input dicts from the full-model inputs."""
    gam = np.asarray(inputs["gamma"], np.float32)
    bet = np.asarray(inputs["beta"], np.float32)
    wip = np.asarray(inputs["in_proj_w"], np.float32)     # (2*DIN, D)
    wop = np.asarray(inputs["out_proj_w"], np.float32)    # (D, DIN)
    opb = np.asarray(inputs["out_proj_b"], np.float32)
    hs = np.asarray(inputs["hidden_states"], np.float32)
    res = np.asarray(inputs["residual"], np.float32)

    conv_w = [np.asarray(inputs["conv_w"], np.float32),
              np.asarray(inputs["conv_w_b"], np.float32)]
    conv_b = [np.asarray(inputs["conv_b"], np.float32),
              np.asarray(inputs["conv_b_b"], np.float32)]
    xw = [np.asarray(inputs["xproj_w"], np.float32),
          np.asarray(inputs["xproj_w_b"], np.float32)]
    dtw = [np.asarray(inputs["dtproj_w"], np.float32),
           np.asarray(inputs["dtproj_w_b"], np.float32)]
    dtb = [np.asarray(inputs["dtproj_b"], np.float32),
           np.asarray(inputs["dtproj_b_b"], np.float32)]
    alog = [np.asarray(inputs["A_log"], np.float32),
            np.asarray(inputs["A_b_log"], np.float32)]
    dp = [np.asarray(inputs["Dp"], np.float32),
          np.asarray(inputs["Dp_b"], np.float32)]

    wip_g = wip * gam[None, :]           # fold gamma
    bias_full = wip @ bet                # fold beta  (2*DIN,)

    i128 = np.eye(128, dtype=np.float16)
    ones = np.ones((128, 1), np.float16)

    # token-major flattening of hs/res: (B, D, L) -> (D, B*L)
    hs_f = hs.transpose(1, 0, 2).reshape(D, TOK)
    res_f = res.transpose(1, 0, 2).reshape(D, TOK)
    hs16 = hs_f.astype(np.float16)
    res16 = res_f.astype(np.float16)

    in_maps = []
    for i in range(NC_):
        ds = slice(i * DL, (i + 1) * DL)
        wxT = wip_g[ds, :].T.astype(np.float16)               # (D, DL)
        wzT = wip_g[DIN + i * DL:DIN + (i + 1) * DL, :].T.astype(np.float16)
        w1s = np.concatenate([
            -wip_g[ds, :].sum(axis=1),
            -wip_g[DIN + i * DL:DIN + (i + 1) * DL, :].sum(axis=1),
        ]).reshape(1, 2 * DL).astype(np.float16)
        bx = bias_full[ds].reshape(DL, 1).astype(np.float32)
        bz = bias_full[DIN + i * DL:DIN + (i + 1) * DL].reshape(DL, 1).astype(np.float32)
        cvd = np.zeros((2, KCV, 2, 128, 128), np.float16)
        cb = np.zeros((2, DL, 1), np.float32)
        xwT = np.zeros((2, DL, DTR + 2 * NST), np.float16)
        dtwT = np.zeros((2, DTR, DL), np.float16)
        dtbv = np.zeros((2, DL, 1), np.float32)
        atab = np.zeros((2, DL, NST), np.float32)
        dpd = np.zeros((2, 2, 128, 128), np.float16)
        for dr in range(2):
            w = conv_w[dr][ds, 0, :]                          # (DL, KCV)
            for j in range(KCV):
                for m in range(2):
                    cvd[dr, j, m] = np.diag(w[m * 128:(m + 1) * 128, j]).astype(np.float16)
            cb[dr] = conv_b[dr][ds].reshape(DL, 1)
            xwT[dr] = xw[dr][:, ds].T.astype(np.float16)      # (DL, 96)
            dtwT[dr] = dtw[dr][ds, :].T.astype(np.float16)    # (DTR, DL)
            dtbv[dr] = dtb[dr][ds].reshape(DL, 1)
            atab[dr] = -np.exp(alog[dr][ds, :])
            for m in range(2):
                dpd[dr, m] = np.diag(dp[dr][ds][m * 128:(m + 1) * 128]).astype(np.float16)
        in_maps.append({
            "hsf": hs16, "resf": res16,
            "hss": np.ascontiguousarray(hs_f[:, i * TSL:(i + 1) * TSL]),
            "ress": np.ascontiguousarray(res_f[:, i * TSL:(i + 1) * TSL]),
            "wxT": wxT, "wzT": wzT, "w1s": w1s, "bx": bx, "bz": bz,
            "convdiag": cvd, "convb": cb,
            "xwT": xwT, "dtwT": dtwT, "dtb": dtbv, "atab": atab,
            "dpdiag": dpd,
            "wopT": wop.T.astype(np.float16),                 # (DIN, D)
            "opb": opb.reshape(D, 1).astype(np.float32),
            "i128": i128, "ones": ones,
        })
    return in_maps


class _Exec:
    """Compile once; run via PJRT shard_map on 8 cores."""

    def __init__(self, nc, n_cores):
        from jax.sharding import Mesh, PartitionSpec
        from jax.experimental.shard_map import shard_map
        bass2jax.install_neuronx_cc_hook()
        self.nc = nc
        self.n = n_cores
        partition_name = nc.partition_id_tensor.name if nc.partition_id_tensor else None
        in_names, out_names, out_avals, zero_outs = [], [], [], []
        for alloc in nc.m.functions[0].allocations:
            if not isinstance(alloc, mybir.MemoryLocationSet):
                continue
            name = alloc.memorylocations[0].name
            if alloc.kind == "ExternalInput":
                if name != partition_name:
                    in_names.append(name)
            elif alloc.kind == "ExternalOutput":
                shape = tuple(alloc.tensor_shape)
                npdt = mybir.dt.np(alloc.dtype)
                out_names.append(name)
                out_avals.append(jax.core.ShapedArray(shape, npdt))
                zero_outs.append(np.zeros(shape, npdt))
        self.in_names, self.out_names = in_names, out_names
        self.out_avals, self.zero_outs = out_avals, zero_outs
        all_in = list(in_names) + list(out_names)
        if partition_name is not None:
            all_in.append(partition_name)

        def _body(*args):
            operands = list(args)
            if partition_name is not None:
                operands.append(bass2jax.partition_id_tensor())
            outs = bass2jax._bass_exec_p.bind(
                *operands,
                out_avals=tuple(out_avals),
                in_names=tuple(all_in),
                out_names=tuple(out_names),
                lowering_input_output_aliases=(),
                sim_require_finite=True,
                sim_require_nnan=True,
                nc=nc,
            )
            return tuple(outs)

        devices = jax.devices()[:n_cores]
        self.mesh = Mesh(np.asarray(devices), ("core",))
        np_ = len(in_names) + len(out_names)
        self.fn = jax.jit(
            shard_map(_body, mesh=self.mesh,
                      in_specs=(PartitionSpec("core"),) * np_,
                      out_specs=(PartitionSpec("core"),) * len(out_names),
                      check_rep=False),
            keep_unused=True)

    def prep(self, in_maps):
        from jax.sharding import NamedSharding, PartitionSpec
        n = self.n
        cat = [np.concatenate([np.asarray(in_maps[c][k]) for c in range(n)], axis=0)
               for k in self.in_names]
        cat += [np.zeros((n * z.shape[0], *z.shape[1:]), z.dtype)
                for z in self.zero_outs]
        sh = NamedSharding(self.mesh, PartitionSpec("core"))
        return [jax.device_put(a, sh) for a in cat]

    def run(self, args):
        outs = self.fn(*args)
        jax.block_until_ready(outs)
        return outs

    def results(self, outs):
        n = self.n
        return [
            {name: np.asarray(outs[i]).reshape(n, *self.out_avals[i].shape)[c]
             for i, name in enumerate(self.out_names)}
            for c in range(n)
        ]


_EXEC = None


def _get_exec():
    global _EXEC
    if _EXEC is None:
        _EXEC = _Exec(build_program(), NC_)
    return _EXEC


def kernel(**inputs):
    e = _get_exec()
    in_maps = _host_prep(inputs)
    res = e.results(e.run(e.prep(in_maps)))
    out = np.zeros((B, D, L), np.float32)
    r = np.zeros((B, D, L), np.float32)
    for i in range(NC_):
        b = i // 4
        l0 = (i % 4) * TSL
        out[b][:, l0:l0 + TSL] = res[i]["o_out"]
        r[b][:, l0:l0 + TSL] = res[i]["r_out"]
    return out, r
